# revision 5
# baseline (speedup 1.0000x reference)
"""CLCE loss kernel v5 for Trainium2 (8 NeuronCores, SPMD).

Device computes the O(N^2 D) part only: a triangle cover of exp(sim) row
sums (per-row-group activation accum_out) plus bf16 column accumulators.
All O(N*C) terms (CE log-softmax, same-class P/slot0) are host-side.

v5 schedule:
- Inputs ride one sync HWDGE queue as 4 slabs of 512KB with 4KB-per-
  partition contiguous descriptors: (h0,k01), (h0,k23), (h1,k01),
  (h1,k23), where h0 = [diag c | c+1] and h1 = [c+2 | c+3]; then x5, wh.
- PSUM tiles are 2-bank [128, 1024] f32: lo_m = {c+1, diag-tri},
  hi_m = {c+2, c+3}.  The m0/m1 lo tiles complete after just the h0
  slabs (1MB), so ScalarE activations start early.
- Diagonal block is computed as an upper triangle at 128-row granularity
  (row group m covers diag cols [m*128, 512)); the mirrored lower part
  comes from a dedicated diag column-accumulator slot.
- One wide Exp+accum_out per tile; b4 is a single no-accum exp whose raw
  bf16 tile is host-reduced for both rows and columns.
- PE warm-up: distinct dummy matmuls (varying W offset/psum slot so they
  survive CSE) start right after a GpSimd memset, ramping the PE clock
  during the DMA fill.
"""

import os
from contextlib import ExitStack

import numpy as np

import concourse.bass as bass
import concourse.tile as tile
from concourse import bacc, mybir
from concourse.bass_utils import run_bass_kernel_spmd

N, D, C = 4096, 1024, 512
TAU = 0.5
LAMBD = 0.5
NCORES = 8
BLK = 512                  # chunk width (rows per core block)
P = 128                    # partitions
KT = D // 256              # 4 DoubleRow contraction super-tiles
MT = BLK // P              # 4 m-tiles (row groups) per core block
S8 = 16.0                  # fp8 pre-scale for the embeddings

_F32 = mybir.dt.float32
_BF16 = mybir.dt.bfloat16
_FP8 = mybir.dt.float8e4
_EXP = mybir.ActivationFunctionType.Exp
_DR = mybir.MatmulPerfMode.DoubleRow

# OUTSB column map (f32, [128, 8])
#  0..7 : sim row sums, col 2*m+h  (h=0 lo: {c+1, diag-tri}; h=1 hi: {c+2, c+3})
NRS = 8

# ET flat layout per row group ([P, 2048] bf16):
#  [0:512)      c+1
#  [512:512+w)  diag-tri cols [m*128, 512), w = 512-128*m
#  [1024:1536)  c+2
#  [1536:2048)  c+3
# ACC flat layout ([P, 2048] bf16): [c+1 | c+2 | c+3 | diag]


def _build_kernel(tc, slabs, x5, wh, out_rs, out_acc, out_b4):
    """slabs: [2, 2, P, 2, 2, 1024] fp8  (h, kpair) -> 4KB/partition slab
    x5:    [P, KT, 2, BLK]  fp8  b4 X chunk (c<4: ch(c+4); c>=4: ch(c))
    wh:    [P, KT, 2, 2*P]  fp8  b4 W half
    out_rs: [P, NRS]  f32; out_acc: [P, 4, BLK] bf16; out_b4: [P, 2, BLK] bf16
    """
    nc = tc.nc
    act_scale = 0.5 * TAU / (S8 * S8)
    with ExitStack() as ctx:
        pers = ctx.enter_context(tc.tile_pool(name="pers", bufs=1))
        psum = ctx.enter_context(
            tc.tile_pool(name="psum", bufs=4, space=bass.MemorySpace.PSUM)
        )

        # h-major so each (h, kpair) DMA lands 4KB contiguous per partition
        BIG = pers.tile([P, 2, KT, 2, 1024], _FP8)
        X5T = pers.tile([P, KT, 2, BLK], _FP8)
        WHT = pers.tile([P, KT, 2, 2 * P], _FP8)
        OUTSB = pers.tile([P, NRS], _F32)
        ACC = pers.tile([P, 4 * BLK], _BF16)
        ET = [pers.tile([P, 4 * BLK], _BF16, name=f"et{m}") for m in range(MT)]
        ETB4 = pers.tile([P, 2, BLK], _BF16)
        bias_s = pers.tile([P, 1], _F32)
        bias_z = pers.tile([P, 1], _F32)
        warm = pers.tile([P, 1], _F32)
        ZW = pers.tile([P, BLK], _BF16)

        # --- input DMAs: one sync HWDGE queue, exact consumption order ---
        # first slab split into k0/k1 pieces so the PE starts sooner
        nc.sync.dma_start(BIG[:, 0, 0, :, :], slabs[0, 0, :, 0])
        nc.sync.dma_start(BIG[:, 0, 1, :, :], slabs[0, 0, :, 1])
        nc.sync.dma_start(BIG[:, 0, 2:4, :, :], slabs[0, 1])
        for kp in range(2):
            nc.sync.dma_start(BIG[:, 1, 2 * kp:2 * kp + 2, :, :],
                              slabs[1, kp])
        nc.sync.dma_start(X5T[:], x5[:])
        nc.sync.dma_start(WHT[:], wh[:])

        # gpsimd wakes earliest: warm-up operand ASAP
        nc.gpsimd.memset(ZW[:], 0.0)
        nc.vector.memset(bias_z[:], 0.0)
        nc.vector.memset(bias_s[:], 0.5 * TAU)
        # diag CS slot: cols [0:128) never written by the adds below
        nc.vector.memset(ACC[:, 3 * BLK:3 * BLK + P], 0.0)

        # scalar: warm activation forces the ACT table load before the chase
        nc.scalar.activation(warm[:], bias_z[:], _EXP, bias=bias_z[:], scale=1.0)

        # chase-phase PSUM tiles (2 banks each)
        RLO = [psum.tile([P, 2 * BLK], _F32, tag="ps", name=f"rlo{m}")
               for m in range(2)]
        RHI = [psum.tile([P, 2 * BLK], _F32, tag="ps", name=f"rhi{m}")
               for m in range(2)]

        # PE warm-up: distinct (W-offset, psum-slot) so CSE keeps them all
        for i in range(6):
            woff = (i % 4) * P
            slot = (i // 4) * BLK
            nc.tensor.matmul(RLO[0][:, slot:slot + BLK],
                             ZW[:, woff:woff + P] if woff + P <= BLK
                             else ZW[:, 0:P],
                             ZW[:], start=True, stop=True)

        def wslice(k, m):
            return BIG[:, 0, k, :, m * P:(m + 1) * P]

        # X operand for chunk j (0=diag c, 1=c+1, 2=c+2, 3=c+3); diag may be
        # narrowed to cols [off, 512)
        def xop(k, j, off=0):
            base = (j % 2) * BLK
            return BIG[:, j // 2, k, :, base + off:base + BLK]

        def mm(ps, k, m, j, off=0):
            nc.tensor.matmul(
                ps, wslice(k, m), xop(k, j, off),
                start=(k == 0), stop=(k == KT - 1), perf_mode=_DR,
            )

        def lo_fill(ps, m, k):
            mm(ps[:, 0:BLK], k, m, 1)                       # c+1
            mm(ps[:, BLK:2 * BLK - m * P], k, m, 0, m * P)  # diag tri

        def hi_fill(ps, m, k):
            mm(ps[:, 0:BLK], k, m, 2)
            mm(ps[:, BLK:2 * BLK], k, m, 3)

        # --- chase: h0 slabs fill lo tiles, h1 slabs fill hi tiles ---
        # m-outer: RLO0 completes first so its act starts ASAP
        for m in range(2):
            for k in range(KT):
                lo_fill(RLO[m], m, k)
        for m in range(2):
            for k in range(KT):
                hi_fill(RHI[m], m, k)

        def act_lo(ps, m):
            w = 2 * BLK - m * P
            nc.scalar.activation(
                ET[m][:, 0:w], ps[:, 0:w], _EXP,
                bias=bias_s[:], scale=act_scale,
                accum_out=OUTSB[:, 2 * m:2 * m + 1],
            )

        def act_hi(ps, m):
            nc.scalar.activation(
                ET[m][:, 2 * BLK:4 * BLK], ps[:], _EXP,
                bias=bias_s[:], scale=act_scale,
                accum_out=OUTSB[:, 2 * m + 1:2 * m + 2],
            )

        act_lo(RLO[0], 0)
        act_lo(RLO[1], 1)
        act_hi(RHI[0], 0)
        act_hi(RHI[1], 1)

        # --- phase 2: row groups m2/m3 from SBUF-resident chunks ---
        def fill_lo(m):
            ps = psum.tile([P, 2 * BLK], _F32, tag="ps")
            for k in range(KT):
                lo_fill(ps, m, k)
            return ps

        def fill_hi(m):
            ps = psum.tile([P, 2 * BLK], _F32, tag="ps")
            for k in range(KT):
                hi_fill(ps, m, k)
            return ps

        p2lo = fill_lo(2)
        act_lo(p2lo, 2)
        p2hi = fill_hi(2)
        act_hi(p2hi, 2)
        # hi tile first: the big ACC [BLK:3BLK] DMA depends on act_hi3, so
        # finishing it earlier drains the output queue before the b4 DMAs
        p3hi = fill_hi(3)
        act_hi(p3hi, 3)
        p3lo = fill_lo(3)
        act_lo(p3lo, 3)

        # --- b4 half-block: W from wh, X from x5, rows mm=0,1 ---
        # halves pipelined: act+DMA of half 0 overlap the half-1 fill
        psb = psum.tile([P, 2 * BLK], _F32, tag="ps")
        for mm_i in range(2):
            for k in range(KT):
                nc.tensor.matmul(
                    psb[:, mm_i * BLK:(mm_i + 1) * BLK],
                    WHT[:, k, :, mm_i * P:(mm_i + 1) * P],
                    X5T[:, k, :, :],
                    start=(k == 0), stop=(k == KT - 1), perf_mode=_DR,
                )
            nc.scalar.activation(ETB4[:, mm_i, :],
                                 psb[:, mm_i * BLK:(mm_i + 1) * BLK], _EXP,
                                 bias=bias_s[:], scale=act_scale)
            nc.sync.dma_start(out_b4[:, mm_i, :], ETB4[:, mm_i, :])

        # --- column accumulators on DVE ---
        # slots: ACC[0:512)=c+1, [512:1536)=c+2|c+3, [1536:2048)=diag
        nc.vector.tensor_add(ACC[:, 0:BLK], ET[0][:, 0:BLK], ET[1][:, 0:BLK])
        nc.vector.tensor_add(ACC[:, BLK:3 * BLK], ET[0][:, 2 * BLK:4 * BLK],
                             ET[1][:, 2 * BLK:4 * BLK])
        # diag strict-upper: m0 cols [128:512) seed the slot, m1/m2 add
        nc.vector.tensor_copy(ACC[:, 3 * BLK + P:4 * BLK],
                              ET[0][:, BLK + P:2 * BLK])
        nc.vector.tensor_add(ACC[:, 3 * BLK + 2 * P:4 * BLK],
                             ACC[:, 3 * BLK + 2 * P:4 * BLK],
                             ET[1][:, BLK + P:2 * BLK - P])
        nc.vector.tensor_add(ACC[:, 0:BLK], ACC[:, 0:BLK], ET[2][:, 0:BLK])
        nc.vector.tensor_add(ACC[:, BLK:3 * BLK], ACC[:, BLK:3 * BLK],
                             ET[2][:, 2 * BLK:4 * BLK])
        nc.vector.tensor_add(ACC[:, 3 * BLK + 3 * P:4 * BLK],
                             ACC[:, 3 * BLK + 3 * P:4 * BLK],
                             ET[2][:, BLK + P:2 * BLK - 2 * P])
        # diag slot final after m2: ship it while the m3 adds run
        nc.sync.dma_start(out_acc[:, 3 * BLK:4 * BLK], ACC[:, 3 * BLK:4 * BLK])
        nc.vector.tensor_add(ACC[:, BLK:3 * BLK], ACC[:, BLK:3 * BLK],
                             ET[3][:, 2 * BLK:4 * BLK])
        nc.sync.dma_start(out_acc[:, BLK:3 * BLK], ACC[:, BLK:3 * BLK])
        nc.vector.tensor_add(ACC[:, 0:BLK], ACC[:, 0:BLK], ET[3][:, 0:BLK])
        nc.sync.dma_start(out_acc[:, 0:BLK], ACC[:, 0:BLK])
        nc.scalar.dma_start(out_rs[:], OUTSB[:])


_NC_CACHE = None


def _get_nc():
    global _NC_CACHE
    if _NC_CACHE is None:
        nc = bacc.Bacc(
            "TRN2", target_bir_lowering=False, debug=False,
            enable_asserts=False, num_devices=NCORES,
        )
        slabs_d = nc.dram_tensor("slabs", [2, 2, P, 2, 2, 1024], _FP8,
                                 kind="ExternalInput")
        x5_d = nc.dram_tensor("x5", [P, KT, 2, BLK], _FP8, kind="ExternalInput")
        wh_d = nc.dram_tensor("wh", [P, KT, 2, 2 * P], _FP8,
                              kind="ExternalInput")
        out_rs_d = nc.dram_tensor("out_rs", [P, NRS], _F32,
                                  kind="ExternalOutput")
        out_acc_d = nc.dram_tensor("out_acc", [P, 4 * BLK], _BF16,
                                   kind="ExternalOutput")
        out_b4_d = nc.dram_tensor("out_b4", [P, 2, BLK], _BF16,
                                  kind="ExternalOutput")
        with tile.TileContext(nc) as tc:
            _build_kernel(
                tc, slabs_d.ap(), x5_d.ap(), wh_d.ap(),
                out_rs_d.ap(), out_acc_d.ap(), out_b4_d.ap(),
            )
        nc.compile()
        _NC_CACHE = nc
    return _NC_CACHE


def _pk(cols):
    """[D, w] fp8 -> [P, KT, 2, w]; contraction index kk*256+128*i+p maps to
    tile element [p, kk, i, :] (DoubleRow pairing, same as the baseline)."""
    w = cols.shape[1]
    return np.ascontiguousarray(
        cols.reshape(KT, 2, P, w).transpose(2, 0, 1, 3)
    )


def _run_device(xnT, trace=False):
    """Run the SPMD kernel; returns T[N] f64 and the raw results."""
    fp8np = mybir.dt.np(_FP8)
    zq = (xnT * S8).astype(np.float32).astype(fp8np)  # [D, N] fp8
    ch = lambda j: zq[:, (j % 8) * BLK:(j % 8) * BLK + BLK]
    in_maps = []
    for c in range(NCORES):
        # slabs[h, kp, p, kk, i, n]: pk_h[p, 2*kp+kk, i, n]
        slabs = np.empty((2, 2, P, 2, 2, 1024), dtype=zq.dtype)
        for h, pair in enumerate(((c, c + 1), (c + 2, c + 3))):
            ph = _pk(np.concatenate([ch(pair[0]), ch(pair[1])], axis=1))
            # ph: [P, KT, 2, 1024] -> [kp, P, kk, i, n]
            slabs[h] = ph.reshape(P, 2, 2, 2, 1024).transpose(1, 0, 2, 3, 4)
        if c < 4:
            x5 = _pk(ch(c + 4))
            whp = _pk(ch(c)[:, 0:2 * P])
        else:
            x5 = _pk(ch(c))
            whp = _pk(ch(c - 4)[:, BLK - 2 * P:BLK])
        in_maps.append({"slabs": np.ascontiguousarray(slabs),
                        "x5": x5, "wh": whp})
    res = run_bass_kernel_spmd(
        _get_nc(), in_maps, core_ids=list(range(NCORES)), trace=trace,
    )
    T = np.zeros(N, np.float64)
    for c, r in enumerate(res.results):
        o = r["out_rs"].astype(np.float64)             # [128, 8]
        acc = r["out_acc"].astype(np.float64).reshape(P, 4, BLK)
        b4 = r["out_b4"].astype(np.float64)            # [128, 2, 512]
        for m in range(MT):
            rows = slice(c * BLK + m * P, c * BLK + (m + 1) * P)
            T[rows] += o[:, 2 * m] + o[:, 2 * m + 1]
        # b4 half-block row sums (host reduction of the raw exp tiles)
        b4rs = b4.sum(axis=2)                          # [128, 2]
        if c < 4:
            p0 = c * BLK
            T[p0:p0 + P] += b4rs[:, 0]
            T[p0 + P:p0 + 2 * P] += b4rs[:, 1]
        else:
            p0 = (c - 4) * BLK
            T[p0 + 2 * P:p0 + 3 * P] += b4rs[:, 0]
            T[p0 + 3 * P:p0 + 4 * P] += b4rs[:, 1]
        # column sums (host-side partition reduction)
        chs = lambda j: slice((j % 8) * BLK, (j % 8) * BLK + BLK)
        T[chs(c + 1)] += acc[:, 0, :].sum(0)
        T[chs(c + 2)] += acc[:, 1, :].sum(0)
        T[chs(c + 3)] += acc[:, 2, :].sum(0)
        T[chs(c)] += acc[:, 3, :].sum(0)               # diag mirrored part
        T[chs(c + 4 if c < 4 else c)] += b4.sum(axis=(0, 1))
    return T, res


def kernel(layer_embeds, y_true, y_pred):
    x = np.asarray(layer_embeds, dtype=np.float32)
    yt = np.asarray(y_true).astype(np.int64)
    yp = np.asarray(y_pred, dtype=np.float64)

    # normalize rows (torch-style eps clip)
    norms = np.maximum(
        np.sqrt((x.astype(np.float64) ** 2).sum(1, keepdims=True)), 1e-8
    )
    xn = (x / norms).astype(np.float32)
    xnT = np.ascontiguousarray(xn.T)  # [D, N]

    trace = bool(int(os.environ.get("CLCE_TRACE", "0")))
    T, res = _run_device(xnT, trace=trace)
    if trace:
        kernel.last_results = res

    # --- host-side small terms ---
    fp8np = mybir.dt.np(_FP8)
    xq = (xn * S8).astype(fp8np).astype(np.float64) / S8  # device-visible xn
    counts = np.bincount(yt, minlength=C)
    P_ = np.zeros(N, np.float64)
    slot0 = np.zeros(N, np.float64)
    for cval in np.unique(yt):
        idx = np.where(yt == cval)[0]
        subq = xq[idx]
        sq = (subq @ subq.T + 1.0) * (0.5 * TAU)   # device-matching sim
        P_[idx] = np.exp(sq).sum(1)
        if len(idx) >= 2:
            sub = xn[idx].astype(np.float64)
            s = (sub @ sub.T + 1.0) * (0.5 * TAU)
            firstpos = np.where(np.arange(len(idx)) == 0, 1, 0)
            slot0[idx] = s[np.arange(len(idx)), firstpos]

    num_neg = N - counts[yt]
    S = T - P_
    Z = (2 * N - 2 - num_neg).astype(np.float64)
    cl = (np.log(np.exp(slot0) + S + Z) - slot0).mean()

    # CE term fully on host (O(N*C), f64-exact log-softmax)
    mp = yp.max(axis=1, keepdims=True)
    lsep = np.log(np.exp(yp - mp).sum(axis=1)) + mp[:, 0]
    ce = (lsep - yp[np.arange(N), yt]).mean()

    loss = LAMBD * cl + (1.0 - LAMBD) * ce
    return np.asarray(loss, dtype=np.float32)
